# revision 63
# baseline (speedup 1.0000x reference)
"""Multi-Head Latent Attention (MLA) forward pass on 8 Trainium2 NeuronCores.

Sharding: num_heads tensor-parallel (2 heads/core) for up-projections,
attention and out-proj; the low-rank down-projections + LayerNorm are
token-parallel (512 tokens/core) followed by on-device AllGathers of the
bf16 latents (kv first, overlapped with the q path). Per-core partial
outputs (out-proj with input-dim-sliced Wout) are summed on the host.

Scheduling: engines execute their queues in order, so the emission order
software-pipelines the work: attention chunk c's key-tile loop carries
the q-path projections for chunk c+1 and the out-projection of chunk
c-1, with a fixed PSUM bank map so phases don't serialize on bank reuse.
Rope's rotate-half branch is a single 128x128 permutation matmul instead
of a second full-rank projection.

Self-contained: hardcodes all shapes from the problem spec.
"""

from contextlib import ExitStack

import numpy as np
import ml_dtypes

import concourse.bass as bass
import concourse.mybir as mybir
import concourse.tile as tile
from concourse import bacc
from concourse.bass_utils import run_bass_kernel_spmd
from concourse.masks import make_identity

# ---- problem dimensions (hardcoded) ----
NCORES = 8
P = 128
B = 2
S = 2048           # sequence length
T = B * S          # total tokens = 4096
D = 2048           # d_model
QR = 1536          # q rank
KVR = 512          # kv rank
H = 16             # heads
HD = 128           # head dim (content)
RD = 64            # rope dim
HLOC = H // NCORES # heads per core = 2
TLOC = T // NCORES # tokens per core = 512
NQ = HLOC * HD     # 256 per-core content out dims
NR = HLOC * RD     # 128 per-core rope out dims
SCALE = (HD + RD) ** -0.5
LN_EPS = 1e-5

BF = mybir.dt.bfloat16
F32 = mybir.dt.float32
AX = mybir.AxisListType
OP = mybir.AluOpType
ACT = mybir.ActivationFunctionType

NKT = S // P       # 16 key tiles per sequence
KQ = QR // P       # 12
KKV = KVR // P     # 4
KX = D // P        # 16
MT = TLOC // P     # 4 token tiles per core
NCH = 8            # token chunks of 512 across T


def build(has_bias: bool):
    nc = bacc.Bacc("TRN2", target_bir_lowering=False, debug=False,
                   num_devices=NCORES, enable_asserts=False)

    def din(name, shape, dt=BF):
        return nc.dram_tensor(name, shape, dt, kind="ExternalInput").ap()

    xt = din("xt", [D, TLOC])
    wq_down = din("wq_down", [D, QR])
    wkv_down = din("wkv_down", [D, KVR])
    gq_up = din("gq_up", [QR, NQ])
    gq_rope = din("gq_rope", [QR, NR])
    gk_up = din("gk_up", [KVR, NQ])
    gk_rope = din("gk_rope", [KVR, NR])
    gv_up = din("gv_up", [KVR, NQ])
    wout = din("wout", [NQ, D])
    cos_t = din("cos_t", [NR, T])
    sin_t = din("sin_t", [NR, T])
    pi_t = din("pi_t", [NR, NR])
    if has_bias:
        bq_up = din("bq_up", [1, NQ])
        bq_rope = din("bq_rope", [1, NR])
        bk_up = din("bk_up", [1, NQ])
        bk_rope = din("bk_rope", [1, NR])
        bv_up = din("bv_up", [1, NQ])
    out_p = nc.dram_tensor("out_p", [T, D], BF, kind="ExternalOutput").ap()

    agi_kv = nc.dram_tensor("agi_kv", [KVR, TLOC], BF).ap()
    ago_kv = nc.dram_tensor("ago_kv", [NCORES * KVR, TLOC], BF,
                            addr_space="Shared").ap()
    agi_q = nc.dram_tensor("agi_q", [QR, TLOC], BF).ap()
    ago_q = nc.dram_tensor("ago_q", [NCORES * QR, TLOC], BF,
                           addr_space="Shared").ap()

    with tile.TileContext(nc) as tc, ExitStack() as stk:
        # ---------------- constants ----------------
        const = stk.enter_context(tc.tile_pool(name="const", bufs=1))
        ident = const.tile([P, P], BF)
        make_identity(nc, ident)
        ones_col = const.tile([P, 1], BF)
        nc.vector.memset(ones_col, 1.0)
        ones_tok = const.tile([1, TLOC], BF)
        nc.vector.memset(ones_tok, 1.0)
        ones_row = const.tile([1, P], BF)
        nc.vector.memset(ones_row, 1.0)
        eps_t = const.tile([P, 1], F32)
        nc.vector.memset(eps_t, LN_EPS)
        # off the sync queue so x/w loads aren't head-blocked at startup
        pi_sb = const.tile([NR, NR], BF)
        nc.scalar.dma_start(pi_sb, pi_t)
        cos_sb = const.tile([NR, T], BF)
        nc.scalar.dma_start(cos_sb, cos_t)
        sin_sb = const.tile([NR, T], BF)
        nc.scalar.dma_start(sin_sb, sin_t)

        # PSUM bank map (8 banks):
        #   sp0,sp1     : paired score tiles, 2 banks each (both heads side
        #                 by side; also phase-1 down accum, kv-path k_up)
        #   o0,o1       : PV accumulators (also rope Pi rotation)
        #   po (bufs=2) : q-path proj + out-proj + denominators + transposes
        psum = stk.enter_context(tc.tile_pool(name="psum", bufs=1, space="PSUM"))
        PBUFS = {"po": 2}

        def pst(tag, shape=None, dt=F32):
            return psum.tile(shape or [P, TLOC], dt, tag=tag, name=tag,
                             bufs=PBUFS.get(tag, 1))

        # ------------- phase 1: down-proj + LN + transpose, kv first -------------
        with (
            tc.tile_pool(name="p1x", bufs=1) as xpool,
            tc.tile_pool(name="p1w", bufs=2) as wpool,
            tc.tile_pool(name="p1c", bufs=1) as cpool,
            tc.tile_pool(name="p1z", bufs=1) as zpool,
            tc.tile_pool(name="p1s", bufs=2) as spool,
        ):
            x_all = xpool.tile([P, KX, TLOC], BF)
            xr = xt.rearrange("(k p) m -> p k m", p=P)

            def x_quarter(q4):
                nc.sync.dma_start(x_all[:, q4 * 4:(q4 + 1) * 4, :],
                                  xr[:, q4 * 4:(q4 + 1) * 4, :])

            z_kv = zpool.tile([P, KKV, TLOC], BF)
            z_q = zpool.tile([P, KQ, TLOC], BF)
            kv_ct = [cpool.tile([P, KVR], F32, name=f"ckv{m}") for m in range(MT)]
            q_ct = [cpool.tile([P, QR], F32, name=f"cq{m}") for m in range(MT)]


            def down_mms(wdram, ctiles, nch, with_x=False):
                w_all = wpool.tile([P, KX, 512], BF, tag="w")
                wr = wdram[:, nch * 512:(nch + 1) * 512].rearrange(
                    "(k p) n -> p k n", p=P)
                for q4 in range(4):
                    if with_x:  # interleave so the k=0 matmul starts earliest
                        x_quarter(q4)
                    nc.sync.dma_start(w_all[:, q4 * 4:(q4 + 1) * 4, :],
                                      wr[:, q4 * 4:(q4 + 1) * 4, :])
                # m-outer so each token tile's result (and its LayerNorm)
                # is ready a quarter-chunk earlier; copies on DVE only so the
                # Scalar queue flows straight to the LayerNorm -> AllGather
                sps = [pst(f"sp{i}", [P, 2 * TLOC]) for i in range(2)]
                for m in range(MT):
                    ps = sps[m // 2][:, (m % 2) * 512:(m % 2 + 1) * 512]
                    for k in range(KX):
                        nc.tensor.matmul(
                            ps, x_all[:, k, m * P:(m + 1) * P],
                            w_all[:, k, :],
                            start=(k == 0), stop=(k == KX - 1))
                    nc.vector.tensor_copy(
                        ctiles[m][:, nch * 512:(nch + 1) * 512], ps)

            def layer_norm_m(rank, cb):
                """LN for one 128-token tile; returns the normalized bf16 cn."""
                sumt = spool.tile([P, 1], F32, tag="sumt")
                nc.vector.reduce_sum(sumt, cb[:], axis=AX.X)
                sqs = spool.tile([P, rank], BF, tag="sqs")
                ss = spool.tile([P, 1], F32, tag="ss")
                nc.scalar.activation(sqs[:], cb[:], ACT.Square,
                                     accum_out=ss[:])
                mu = spool.tile([P, 1], F32, tag="mu")
                nc.vector.tensor_scalar_mul(mu, sumt, 1.0 / rank)
                musq = spool.tile([P, 1], F32, tag="musq")
                nc.vector.tensor_mul(musq, mu, mu)
                var = spool.tile([P, 1], F32, tag="var")
                nc.vector.tensor_scalar_mul(var, ss, 1.0 / rank)
                nc.vector.tensor_sub(var, var, musq)
                sd = spool.tile([P, 1], F32, tag="sd")
                nc.scalar.activation(sd, var, ACT.Sqrt, bias=eps_t[:])
                inv = spool.tile([P, 1], F32, tag="inv")
                nc.vector.reciprocal(inv, sd)
                cn = spool.tile([P, rank], BF, tag="cn")
                nc.vector.tensor_scalar(
                    cn[:], cb[:], scalar1=mu, scalar2=inv,
                    op0=OP.subtract, op1=OP.mult)
                return cn

            def transpose_m(nk, cn, zdst, m):
                for f in range(nk):
                    tps = pst("po", [P, P], BF)
                    nc.tensor.transpose(tps, cn[:, f * P:(f + 1) * P], ident)
                    if f % 2 == 0:
                        nc.vector.tensor_copy(
                            zdst[:, f, m * P:(m + 1) * P], tps)
                    else:
                        nc.scalar.activation(
                            zdst[:, f, m * P:(m + 1) * P], tps, ACT.Copy)

            def down_finish(rank, ctiles, zdst):
                nk = rank // P
                for m in range(MT):
                    cn = layer_norm_m(rank, ctiles[m])
                    transpose_m(nk, cn, zdst, m)

            down_mms(wkv_down, kv_ct, 0, with_x=True)
            down_finish(KVR, kv_ct, z_kv)
            nc.scalar.dma_start(
                agi_kv.rearrange("(k p) m -> p k m", p=P), z_kv[:])
            nc.gpsimd.collective_compute(
                "AllGather", OP.bypass,
                ins=[agi_kv[:]], outs=[ago_kv[:]],
                replica_groups=[list(range(NCORES))])

            # q matmuls keep PE busy while kv LayerNorm/AllGather run
            down_mms(wq_down, q_ct, 0)
            down_mms(wq_down, q_ct, 1)
            down_mms(wq_down, q_ct, 2)
            down_finish(QR, q_ct, z_q)
            nc.scalar.dma_start(
                agi_q.rearrange("(k p) m -> p k m", p=P), z_q[:])
            nc.gpsimd.collective_compute(
                "AllGather", OP.bypass,
                ins=[agi_q[:]], outs=[ago_q[:]],
                replica_groups=[list(range(NCORES))])

        # ---------------- phase 2/3 tiles ----------------
        wu = stk.enter_context(tc.tile_pool(name="wu", bufs=1))
        qkv = stk.enter_context(tc.tile_pool(name="qkv", bufs=1))

        def load_w(dram, rows, cols):
            if rows < P:
                t = wu.tile([rows, 1, cols], BF, name="w_" + dram.tensor.name)
                nc.sync.dma_start(t[:, 0, :], dram[:, :])
                return t
            t = wu.tile([P, rows // P, cols], BF, name="w_" + dram.tensor.name)
            nc.sync.dma_start(t[:], dram.rearrange("(k p) n -> p k n", p=P))
            return t

        gqu_t = load_w(gq_up, QR, NQ)
        gqr_t = load_w(gq_rope, QR, NR)
        gku_t = load_w(gk_up, KVR, NQ)
        gkr_t = load_w(gk_rope, KVR, NR)
        gvu_t = load_w(gv_up, KVR, NQ)
        wout_t = load_w(wout, NQ, D)
        if has_bias:
            bqu_t = load_w(bq_up, 1, NQ)
            bqr_t = load_w(bq_rope, 1, NR)
            bku_t = load_w(bk_up, 1, NQ)
            bkr_t = load_w(bk_rope, 1, NR)
            bvu_t = load_w(bv_up, 1, NQ)

        qc_sb = [qkv.tile([P, T], BF, name=f"qc{m}") for m in range(HLOC)]
        kc_sb = [qkv.tile([P, T], BF, name=f"kc{m}") for m in range(HLOC)]
        qr_sb = qkv.tile([NR, T], BF, name="qr")
        kr_sb = qkv.tile([NR, T], BF, name="kr")
        v_sb = qkv.tile([P, T // P, NQ], BF, name="v")
        oT_sb = [qkv.tile([P, T], BF, name=f"oT{m}") for m in range(HLOC)]

        zpool = stk.enter_context(tc.tile_pool(name="p2z", bufs=2))
        rtmp = stk.enter_context(tc.tile_pool(name="p2t", bufs=2))
        apool = stk.enter_context(tc.tile_pool(name="p3s", bufs=2))
        ppool = stk.enter_context(tc.tile_pool(name="p3p", bufs=5))
        opool = stk.enter_context(tc.tile_pool(name="p3o", bufs=3))

        def proj(zt, nk, wt, mcol0, mcols, btile, ptag):
            ps = pst(ptag, [mcols, TLOC])
            for k in range(nk):
                nc.tensor.matmul(
                    ps, wt[:, k, mcol0:mcol0 + mcols], zt[:, k, :],
                    start=(k == 0), stop=(k == nk - 1 and btile is None))
            if btile is not None:
                nc.tensor.matmul(ps, btile[:, 0, mcol0:mcol0 + mcols],
                                 ones_tok[:], start=False, stop=True)
            return ps

        def rope_pair(zt, nk, wr, br, dst, csl, ptag_a, ptag_b):
            """dst[:, csl] = p*cos + rot(p)*sin, rot via Pi matmul."""
            psa = proj(zt, nk, wr, 0, NR, br, ptag_a)
            p_sb = rtmp.tile([NR, TLOC], BF, tag="p_sb", name="p_sb")
            nc.vector.tensor_copy(p_sb, psa)
            psb = pst(ptag_b, [NR, TLOC])
            nc.tensor.matmul(psb, pi_sb, p_sb, start=True, stop=True)
            t1 = rtmp.tile([NR, TLOC], BF, tag="t1", name="t1")
            nc.vector.tensor_mul(t1[:], p_sb[:], cos_sb[:, csl])
            t2 = rtmp.tile([NR, TLOC], BF, tag="t2", name="t2")
            nc.vector.tensor_mul(t2[:], psb[:], sin_sb[:, csl])
            nc.vector.tensor_add(dst[:, csl], t1[:], t2[:])

        # ------------- phase 2: kv path for all chunks -------------
        for c in range(NCH):
            csl = slice(c * TLOC, (c + 1) * TLOC)
            zkv = zpool.tile([P, KKV, TLOC], BF, tag="zkv", name="zkv")
            nc.sync.dma_start(
                zkv[:], ago_kv[c * KVR:(c + 1) * KVR, :]
                .rearrange("(k p) m -> p k m", p=P))
            spk = pst(f"sp{c % 2}", [P, 2 * TLOC])
            for m in range(HLOC):
                ps = spk[:, m * TLOC:(m + 1) * TLOC]
                for k in range(KKV):
                    nc.tensor.matmul(
                        ps, gku_t[:, k, m * P:(m + 1) * P], zkv[:, k, :],
                        start=(k == 0),
                        stop=(k == KKV - 1 and not has_bias))
                if has_bias:
                    nc.tensor.matmul(ps, bku_t[:, 0, m * P:(m + 1) * P],
                                     ones_tok[:], start=False, stop=True)
                nc.vector.tensor_copy(kc_sb[m][:, csl], ps)
            rope_pair(zkv, KKV, gkr_t,
                      bkr_t if has_bias else None, kr_sb, csl, "o0", "o1")
            for mt in range(MT):
                psv = pst("o1" if mt % 2 else "o0", [P, NQ])
                for k in range(KKV):
                    nc.tensor.matmul(
                        psv, zkv[:, k, mt * P:(mt + 1) * P], gvu_t[:, k, :],
                        start=(k == 0), stop=(k == KKV - 1 and not has_bias))
                if has_bias:
                    nc.tensor.matmul(psv, ones_row[:, :P], bvu_t[:, 0, :],
                                     start=False, stop=True)
                if mt % 2 == 0:
                    nc.vector.tensor_copy(v_sb[:, c * MT + mt, :], psv)
                else:
                    nc.scalar.activation(v_sb[:, c * MT + mt, :], psv, ACT.Copy)

        # ------------- q-path chunk emitters (interleaved into phase 3) ------
        def qpath_dma(c):
            # single sync-queue DMA: a second half on the scalar queue would
            # head-block the softmax exps behind it on its ring-buffer WAR
            zq = zpool.tile([P, KQ, TLOC], BF, tag="zq", name="zq")
            nc.sync.dma_start(
                zq[:], ago_q[c * QR:(c + 1) * QR, :]
                .rearrange("(k p) m -> p k m", p=P))
            return zq

        def qpath_pieces(c, zq):
            """Yield thunks: q up-proj + rope for chunk c, small PE pieces."""
            csl = slice(c * TLOC, (c + 1) * TLOC)

            def up(m):
                ps = proj(zq, KQ, gqu_t, m * P, P,
                          bqu_t if has_bias else None, "po")
                nc.vector.tensor_copy(qc_sb[m][:, csl], ps)

            rst = {}

            def rope_a():
                psa = proj(zq, KQ, gqr_t, 0, NR,
                           bqr_t if has_bias else None, "po")
                p_sb = rtmp.tile([NR, TLOC], BF, tag="p_sb", name="p_sb")
                nc.vector.tensor_copy(p_sb, psa)
                rst["p"] = p_sb

            def rope_b():
                p_sb = rst.pop("p")
                psb = pst("po", [NR, TLOC])
                nc.tensor.matmul(psb, pi_sb, p_sb, start=True, stop=True)
                t1 = rtmp.tile([NR, TLOC], BF, tag="t1", name="t1")
                nc.vector.tensor_mul(t1[:], p_sb[:], cos_sb[:, csl])
                t2 = rtmp.tile([NR, TLOC], BF, tag="t2", name="t2")
                nc.vector.tensor_mul(t2[:], psb[:], sin_sb[:, csl])
                nc.vector.tensor_add(qr_sb[:, csl], t1[:], t2[:])

            yield lambda: up(0)
            yield lambda: up(1)
            yield rope_a
            yield rope_b

        # ------------- phase 3: attention, software-pipelined -------------
        # chunk c loop body also emits: out-proj of chunk c-1, q-path of c+1
        def outproj_pieces(c):
            # fine-grained: one (token tile, d-column) piece per yield, so
            # the DVE casts never burst and starve the softmax acc adds
            tok0 = c * TLOC
            ots = {}
            for mt4 in range(MT):
                for nch in range(D // 512):
                    def run(mt4=mt4, nch=nch):
                        mt = (tok0 // P) + mt4
                        if nch == 0:
                            ots[mt4] = opool.tile([P, D], BF, tag="ot",
                                                  name="ot")
                        ot = ots[mt4]
                        po = pst("po", [P, 512])
                        for k2 in range(HLOC):
                            nc.tensor.matmul(
                                po,
                                oT_sb[k2][:, mt * P:(mt + 1) * P],
                                wout_t[:, k2, nch * 512:(nch + 1) * 512],
                                start=(k2 == 0), stop=(k2 == HLOC - 1))
                        # DVE only: ACT copies here would delay the exps
                        # queued behind them and stall the PV pipeline
                        nc.vector.tensor_copy(
                            ot[:, nch * 512:(nch + 1) * 512], po)
                        if nch == D // 512 - 1:
                            nc.gpsimd.dma_start(
                                out_p[mt * P:(mt + 1) * P, :], ot[:])
                    yield run

        def finish_pieces(c):
            """Softmax denominator + normalization for chunk c (after s-loop)."""
            tsl = slice(c * TLOC, (c + 1) * TLOC)
            for h in range(HLOC):
                def run(h=h):
                    den = pst("po", [1, TLOC])
                    nc.tensor.matmul(
                        den, ones_col[:],
                        acc[c % 2][:, h * TLOC:(h + 1) * TLOC],
                        start=True, stop=True)
                    denf = apool.tile([1, TLOC], F32, tag="denf", name="denf")
                    nc.vector.tensor_copy(denf, den)
                    rdf = apool.tile([1, TLOC], F32, tag="rdf", name="rdf")
                    rsc = apool.tile([1, TLOC], F32, tag="rsc", name="rsc")
                    nc.vector.reciprocal_approx_accurate(rdf[:], denf[:], rsc[:])
                    rb = apool.tile([P, TLOC], F32, tag=f"rb{h}", name="rb")
                    nc.gpsimd.partition_broadcast(rb[:], rdf[:])
                    nc.vector.tensor_mul(oT_sb[h][:, tsl], o_ps[c % 2][h][:],
                                         rb[:])
                yield run

        # double-buffered per-chunk state (chunk c vs c-1 overlap);
        # one fused accumulator per chunk parity (both heads side by side)
        acc = [apool.tile([P, 2 * TLOC], BF, tag=f"acc{pp}", name=f"acc{pp}",
                          bufs=1) for pp in range(2)]
        o_ps = [None, None]

        zq_cur = qpath_dma(0)
        for piece in qpath_pieces(0, zq_cur):
            piece()

        for c in range(NCH):
            b, qch = divmod(c, S // TLOC)
            tok0 = c * TLOC
            tsl = slice(tok0, tok0 + TLOC)
            o_ps[c % 2] = [pst(f"o{h}", [P, TLOC]) for h in range(HLOC)]

            # queue of deferred emissions spread across the s-loop
            extra = []
            if c + 1 < NCH:
                zq_nxt = qpath_dma(c + 1)
                extra.extend(qpath_pieces(c + 1, zq_nxt))
            if c > 0:
                extra.extend(outproj_pieces(c - 1))

            pts = {}
            for s in range(NKT + 2):
                kt0 = b * S + s * P
                if s < NKT:
                    # one 2-bank score tile per step (both heads side by
                    # side) -> a single fused exp over 1024 columns
                    stp = pst(f"sp{s % 2}", [P, 2 * TLOC])
                    for h in range(HLOC):
                        nc.tensor.matmul(
                            stp[:, h * TLOC:(h + 1) * TLOC],
                            kc_sb[h][:, kt0:kt0 + P], qc_sb[h][:, tsl],
                            start=True, stop=False)
                    for h in range(HLOC):
                        nc.tensor.matmul(
                            stp[:, h * TLOC:(h + 1) * TLOC],
                            kr_sb[h * RD:(h + 1) * RD, kt0:kt0 + P],
                            qr_sb[h * RD:(h + 1) * RD, tsl],
                            start=False, stop=True)
                    pt = ppool.tile([P, 2 * TLOC], BF, tag="pt", name="pt")
                    nc.scalar.activation(pt[:], stp[:], ACT.Exp)
                    pts[s] = pt
                    # one fused accumulator add per step (both heads)
                    if s == 0:
                        nc.vector.tensor_copy(acc[c % 2][:], pt[:])
                    else:
                        nc.vector.tensor_add(acc[c % 2][:], acc[c % 2][:],
                                             pt[:])
                # PV lagged two steps so exp(sp) is already done when it
                # issues -> no PE stall, weight loads stay hidden
                if s > 1:
                    sp = s - 2
                    ptp = pts.pop(sp)
                    for h in range(HLOC):
                        nc.tensor.matmul(
                            o_ps[c % 2][h],
                            v_sb[:, b * NKT + sp, h * P:(h + 1) * P],
                            ptp[:, h * TLOC:(h + 1) * TLOC],
                            start=(sp == 0), stop=(sp == NKT - 1))
                # spread deferred q-path / out-proj emissions across steps
                if s >= 2 and extra:
                    extra.pop(0)()
                if s >= 8 and extra:
                    extra.pop(0)()
            while extra:
                extra.pop(0)()
            for piece in finish_pieces(c):
                piece()

        for piece in outproj_pieces(NCH - 1):
            piece()

    nc.compile()
    return nc


_BUILD_CACHE = {}


def _get_nc(has_bias: bool):
    if has_bias not in _BUILD_CACHE:
        _BUILD_CACHE[has_bias] = build(has_bias)
    return _BUILD_CACHE[has_bias]


def _bf(a):
    return np.ascontiguousarray(a).astype(ml_dtypes.bfloat16)


def _prep_in_maps(x, Wq_down, q_gamma, q_beta, Wq_up, Wq_rope,
                  Wkv_down, kv_gamma, kv_beta, Wk_up, Wv_up, Wk_rope, Wout):
    x = np.asarray(x, dtype=np.float32)
    xT = np.ascontiguousarray(x.reshape(T, D).T)  # [D, T]

    # rope rotate-half permutation, per 64-dim head block (2 local heads)
    Pi1 = np.zeros((RD, RD), np.float32)
    for i in range(RD // 2):
        Pi1[RD // 2 + i, i] = -1.0
        Pi1[i, RD // 2 + i] = 1.0
    Pi = np.zeros((NR, NR), np.float32)
    Pi[:RD, :RD] = Pi1
    Pi[RD:, RD:] = Pi1

    # rope tables, feature-major, duplicated for the 2 local heads
    inv_freq = 1.0 / (10000.0 ** (np.arange(0, RD, 2, dtype=np.float32) / RD))
    pos = (np.arange(T) % S).astype(np.float32)
    freqs = pos[:, None] * inv_freq[None, :]          # [T, 32]
    emb = np.concatenate([freqs, freqs], axis=1)      # [T, 64]
    cosT = np.ascontiguousarray(np.cos(emb).T)        # [64, T]
    sinT = np.ascontiguousarray(np.sin(emb).T)
    cos2 = np.concatenate([cosT, cosT], axis=0)       # [128, T]
    sin2 = np.concatenate([sinT, sinT], axis=0)

    q_gamma = np.asarray(q_gamma, np.float32)
    q_beta = np.asarray(q_beta, np.float32)
    kv_gamma = np.asarray(kv_gamma, np.float32)
    kv_beta = np.asarray(kv_beta, np.float32)
    has_bias = bool(np.any(q_beta) or np.any(kv_beta))

    Wq_up_h = np.asarray(Wq_up, np.float32).reshape(QR, H, HD)
    Wq_rope_h = np.asarray(Wq_rope, np.float32).reshape(QR, H, RD)
    Wk_up_h = np.asarray(Wk_up, np.float32).reshape(KVR, H, HD)
    Wk_rope_h = np.asarray(Wk_rope, np.float32).reshape(KVR, H, RD)
    Wv_up_h = np.asarray(Wv_up, np.float32).reshape(KVR, H, HD)
    Wout_h = np.asarray(Wout, np.float32).reshape(H, HD, D)

    in_maps = []
    for c in range(NCORES):
        hs = slice(HLOC * c, HLOC * (c + 1))
        wq_up_s = Wq_up_h[:, hs].reshape(QR, NQ)
        wq_rope_s = Wq_rope_h[:, hs].reshape(QR, NR)
        wk_up_s = Wk_up_h[:, hs].reshape(KVR, NQ)
        wk_rope_s = Wk_rope_h[:, hs].reshape(KVR, NR)
        wv_up_s = Wv_up_h[:, hs].reshape(KVR, NQ)
        wout_s = Wout_h[hs].reshape(NQ, D)

        gq_up = q_gamma[:, None] * wq_up_s * SCALE
        gq_rope = q_gamma[:, None] * wq_rope_s * SCALE
        gk_up = kv_gamma[:, None] * wk_up_s
        gk_rope = kv_gamma[:, None] * wk_rope_s
        gv_up = kv_gamma[:, None] * wv_up_s

        m = {
            "xt": _bf(xT[:, c * TLOC:(c + 1) * TLOC]),
            "wq_down": _bf(Wq_down),
            "wkv_down": _bf(Wkv_down),
            "gq_up": _bf(gq_up),
            "gq_rope": _bf(gq_rope),
            "gk_up": _bf(gk_up),
            "gk_rope": _bf(gk_rope),
            "gv_up": _bf(gv_up),
            "wout": _bf(wout_s),
            "cos_t": _bf(cos2),
            "sin_t": _bf(sin2),
            "pi_t": _bf(Pi),
        }
        if has_bias:
            m["bq_up"] = _bf((q_beta @ wq_up_s * SCALE)[None, :])
            m["bq_rope"] = _bf((q_beta @ wq_rope_s * SCALE)[None, :])
            m["bk_up"] = _bf((kv_beta @ wk_up_s)[None, :])
            m["bk_rope"] = _bf((kv_beta @ wk_rope_s)[None, :])
            m["bv_up"] = _bf((kv_beta @ wv_up_s)[None, :])
        in_maps.append(m)
    return in_maps, has_bias


def kernel(**inputs):
    in_maps, has_bias = _prep_in_maps(**inputs)
    nc = _get_nc(has_bias)
    res = run_bass_kernel_spmd(nc, in_maps, list(range(NCORES)))
    out = res.results[0]["out_p"].astype(np.float32)
    for c in range(1, NCORES):
        out = out + res.results[c]["out_p"].astype(np.float32)
    return out.reshape(B, S, D)


# revision 64
# speedup vs baseline: 1.0159x; 1.0159x over previous
"""Multi-Head Latent Attention (MLA) forward pass on 8 Trainium2 NeuronCores.

Sharding: num_heads tensor-parallel (2 heads/core) for up-projections,
attention and out-proj; the low-rank down-projections + LayerNorm are
token-parallel (512 tokens/core) followed by on-device AllGathers of the
bf16 latents (kv first, overlapped with the q path). Per-core partial
outputs (out-proj with input-dim-sliced Wout) are summed on the host.

Scheduling: engines execute their queues in order, so the emission order
software-pipelines the work: attention chunk c's key-tile loop carries
the q-path projections for chunk c+1 and the out-projection of chunk
c-1, with a fixed PSUM bank map so phases don't serialize on bank reuse.
Rope's rotate-half branch is a single 128x128 permutation matmul instead
of a second full-rank projection.

Self-contained: hardcodes all shapes from the problem spec.
"""

from contextlib import ExitStack

import numpy as np
import ml_dtypes

import concourse.bass as bass
import concourse.mybir as mybir
import concourse.tile as tile
from concourse import bacc
from concourse.bass_utils import run_bass_kernel_spmd
from concourse.masks import make_identity

# ---- problem dimensions (hardcoded) ----
NCORES = 8
P = 128
B = 2
S = 2048           # sequence length
T = B * S          # total tokens = 4096
D = 2048           # d_model
QR = 1536          # q rank
KVR = 512          # kv rank
H = 16             # heads
HD = 128           # head dim (content)
RD = 64            # rope dim
HLOC = H // NCORES # heads per core = 2
TLOC = T // NCORES # tokens per core = 512
NQ = HLOC * HD     # 256 per-core content out dims
NR = HLOC * RD     # 128 per-core rope out dims
SCALE = (HD + RD) ** -0.5
LN_EPS = 1e-5

BF = mybir.dt.bfloat16
F32 = mybir.dt.float32
AX = mybir.AxisListType
OP = mybir.AluOpType
ACT = mybir.ActivationFunctionType

NKT = S // P       # 16 key tiles per sequence
KQ = QR // P       # 12
KKV = KVR // P     # 4
KX = D // P        # 16
MT = TLOC // P     # 4 token tiles per core
NCH = 8            # token chunks of 512 across T


def build(has_bias: bool):
    nc = bacc.Bacc("TRN2", target_bir_lowering=False, debug=False,
                   num_devices=NCORES, enable_asserts=False)

    def din(name, shape, dt=BF):
        return nc.dram_tensor(name, shape, dt, kind="ExternalInput").ap()

    xt = din("xt", [D, TLOC])
    wq_down = din("wq_down", [D, QR])
    wkv_down = din("wkv_down", [D, KVR])
    gq_up = din("gq_up", [QR, NQ])
    gq_rope = din("gq_rope", [QR, NR])
    gk_up = din("gk_up", [KVR, NQ])
    gk_rope = din("gk_rope", [KVR, NR])
    gv_up = din("gv_up", [KVR, NQ])
    wout = din("wout", [NQ, D])
    cos_t = din("cos_t", [NR, T])
    sin_t = din("sin_t", [NR, T])
    pi_t = din("pi_t", [NR, NR])
    if has_bias:
        bq_up = din("bq_up", [1, NQ])
        bq_rope = din("bq_rope", [1, NR])
        bk_up = din("bk_up", [1, NQ])
        bk_rope = din("bk_rope", [1, NR])
        bv_up = din("bv_up", [1, NQ])
    out_p = nc.dram_tensor("out_p", [T, D], BF, kind="ExternalOutput").ap()

    agi_kv = nc.dram_tensor("agi_kv", [KVR, TLOC], BF).ap()
    ago_kv = nc.dram_tensor("ago_kv", [NCORES * KVR, TLOC], BF,
                            addr_space="Shared").ap()
    agi_q = nc.dram_tensor("agi_q", [QR, TLOC], BF).ap()
    ago_q = nc.dram_tensor("ago_q", [NCORES * QR, TLOC], BF,
                           addr_space="Shared").ap()

    with tile.TileContext(nc) as tc, ExitStack() as stk:
        # ---------------- constants ----------------
        const = stk.enter_context(tc.tile_pool(name="const", bufs=1))
        ident = const.tile([P, P], BF)
        make_identity(nc, ident)
        ones_col = const.tile([P, 1], BF)
        nc.vector.memset(ones_col, 1.0)
        ones_tok = const.tile([1, TLOC], BF)
        nc.vector.memset(ones_tok, 1.0)
        ones_row = const.tile([1, P], BF)
        nc.vector.memset(ones_row, 1.0)
        eps_t = const.tile([P, 1], F32)
        nc.vector.memset(eps_t, LN_EPS)
        # off the sync queue so x/w loads aren't head-blocked at startup
        pi_sb = const.tile([NR, NR], BF)
        nc.scalar.dma_start(pi_sb, pi_t)
        cos_sb = const.tile([NR, T], BF)
        nc.scalar.dma_start(cos_sb, cos_t)
        sin_sb = const.tile([NR, T], BF)
        nc.scalar.dma_start(sin_sb, sin_t)

        # PSUM bank map (8 banks):
        #   sp0,sp1     : paired score tiles, 2 banks each (both heads side
        #                 by side; also phase-1 down accum, kv-path k_up)
        #   o0,o1       : PV accumulators (also rope Pi rotation)
        #   po (bufs=2) : q-path proj + out-proj + denominators + transposes
        psum = stk.enter_context(tc.tile_pool(name="psum", bufs=1, space="PSUM"))
        PBUFS = {"po": 2}

        def pst(tag, shape=None, dt=F32):
            return psum.tile(shape or [P, TLOC], dt, tag=tag, name=tag,
                             bufs=PBUFS.get(tag, 1))

        # ------------- phase 1: down-proj + LN + transpose, kv first -------------
        with (
            tc.tile_pool(name="p1x", bufs=1) as xpool,
            tc.tile_pool(name="p1w", bufs=2) as wpool,
            tc.tile_pool(name="p1c", bufs=1) as cpool,
            tc.tile_pool(name="p1z", bufs=1) as zpool,
            tc.tile_pool(name="p1s", bufs=2) as spool,
        ):
            x_all = xpool.tile([P, KX, TLOC], BF)
            xr = xt.rearrange("(k p) m -> p k m", p=P)

            def x_quarter(q4):
                nc.sync.dma_start(x_all[:, q4 * 4:(q4 + 1) * 4, :],
                                  xr[:, q4 * 4:(q4 + 1) * 4, :])

            z_kv = zpool.tile([P, KKV, TLOC], BF)
            z_q = zpool.tile([P, KQ, TLOC], BF)
            kv_ct = [cpool.tile([P, KVR], F32, name=f"ckv{m}") for m in range(MT)]
            q_ct = [cpool.tile([P, QR], F32, name=f"cq{m}") for m in range(MT)]


            def down_mms(wdram, ctiles, nch, with_x=False):
                w_all = wpool.tile([P, KX, 512], BF, tag="w")
                wr = wdram[:, nch * 512:(nch + 1) * 512].rearrange(
                    "(k p) n -> p k n", p=P)
                for q4 in range(4):
                    if with_x:  # interleave so the k=0 matmul starts earliest
                        x_quarter(q4)
                    nc.sync.dma_start(w_all[:, q4 * 4:(q4 + 1) * 4, :],
                                      wr[:, q4 * 4:(q4 + 1) * 4, :])
                # m-outer so each token tile's result (and its LayerNorm)
                # is ready a quarter-chunk earlier; copies on DVE only so the
                # Scalar queue flows straight to the LayerNorm -> AllGather
                sps = [pst(f"sp{i}", [P, 2 * TLOC]) for i in range(2)]
                for m in range(MT):
                    ps = sps[m // 2][:, (m % 2) * 512:(m % 2 + 1) * 512]
                    for k in range(KX):
                        nc.tensor.matmul(
                            ps, x_all[:, k, m * P:(m + 1) * P],
                            w_all[:, k, :],
                            start=(k == 0), stop=(k == KX - 1))
                    nc.vector.tensor_copy(
                        ctiles[m][:, nch * 512:(nch + 1) * 512], ps)

            def layer_norm_m(rank, cb):
                """LN for one 128-token tile; returns the normalized bf16 cn."""
                sumt = spool.tile([P, 1], F32, tag="sumt")
                nc.vector.reduce_sum(sumt, cb[:], axis=AX.X)
                sqs = spool.tile([P, rank], BF, tag="sqs")
                ss = spool.tile([P, 1], F32, tag="ss")
                nc.scalar.activation(sqs[:], cb[:], ACT.Square,
                                     accum_out=ss[:])
                mu = spool.tile([P, 1], F32, tag="mu")
                nc.vector.tensor_scalar_mul(mu, sumt, 1.0 / rank)
                musq = spool.tile([P, 1], F32, tag="musq")
                nc.vector.tensor_mul(musq, mu, mu)
                var = spool.tile([P, 1], F32, tag="var")
                nc.vector.tensor_scalar_mul(var, ss, 1.0 / rank)
                nc.vector.tensor_sub(var, var, musq)
                sd = spool.tile([P, 1], F32, tag="sd")
                nc.scalar.activation(sd, var, ACT.Sqrt, bias=eps_t[:])
                inv = spool.tile([P, 1], F32, tag="inv")
                nc.vector.reciprocal(inv, sd)
                cn = spool.tile([P, rank], BF, tag="cn")
                nc.vector.tensor_scalar(
                    cn[:], cb[:], scalar1=mu, scalar2=inv,
                    op0=OP.subtract, op1=OP.mult)
                return cn

            def transpose_m(nk, cn, zdst, m):
                for f in range(nk):
                    tps = pst("po", [P, P], BF)
                    nc.tensor.transpose(tps, cn[:, f * P:(f + 1) * P], ident)
                    if f % 2 == 0:
                        nc.vector.tensor_copy(
                            zdst[:, f, m * P:(m + 1) * P], tps)
                    else:
                        nc.scalar.activation(
                            zdst[:, f, m * P:(m + 1) * P], tps, ACT.Copy)

            def down_finish(rank, ctiles, zdst):
                nk = rank // P
                for m in range(MT):
                    cn = layer_norm_m(rank, ctiles[m])
                    transpose_m(nk, cn, zdst, m)

            down_mms(wkv_down, kv_ct, 0, with_x=True)
            down_finish(KVR, kv_ct, z_kv)
            nc.scalar.dma_start(
                agi_kv.rearrange("(k p) m -> p k m", p=P), z_kv[:])
            nc.gpsimd.collective_compute(
                "AllGather", OP.bypass,
                ins=[agi_kv[:]], outs=[ago_kv[:]],
                replica_groups=[list(range(NCORES))])

            # q matmuls keep PE busy while kv LayerNorm/AllGather run
            down_mms(wq_down, q_ct, 0)
            down_mms(wq_down, q_ct, 1)
            down_mms(wq_down, q_ct, 2)
            down_finish(QR, q_ct, z_q)
            nc.scalar.dma_start(
                agi_q.rearrange("(k p) m -> p k m", p=P), z_q[:])
            nc.gpsimd.collective_compute(
                "AllGather", OP.bypass,
                ins=[agi_q[:]], outs=[ago_q[:]],
                replica_groups=[list(range(NCORES))])

        # ---------------- phase 2/3 tiles ----------------
        wu = stk.enter_context(tc.tile_pool(name="wu", bufs=1))
        qkv = stk.enter_context(tc.tile_pool(name="qkv", bufs=1))

        def load_w(dram, rows, cols):
            if rows < P:
                t = wu.tile([rows, 1, cols], BF, name="w_" + dram.tensor.name)
                nc.sync.dma_start(t[:, 0, :], dram[:, :])
                return t
            t = wu.tile([P, rows // P, cols], BF, name="w_" + dram.tensor.name)
            nc.sync.dma_start(t[:], dram.rearrange("(k p) n -> p k n", p=P))
            return t

        gqu_t = load_w(gq_up, QR, NQ)
        gqr_t = load_w(gq_rope, QR, NR)
        gku_t = load_w(gk_up, KVR, NQ)
        gkr_t = load_w(gk_rope, KVR, NR)
        gvu_t = load_w(gv_up, KVR, NQ)
        wout_t = load_w(wout, NQ, D)
        if has_bias:
            bqu_t = load_w(bq_up, 1, NQ)
            bqr_t = load_w(bq_rope, 1, NR)
            bku_t = load_w(bk_up, 1, NQ)
            bkr_t = load_w(bk_rope, 1, NR)
            bvu_t = load_w(bv_up, 1, NQ)

        qc_sb = [qkv.tile([P, T], BF, name=f"qc{m}") for m in range(HLOC)]
        kc_sb = [qkv.tile([P, T], BF, name=f"kc{m}") for m in range(HLOC)]
        qr_sb = qkv.tile([NR, T], BF, name="qr")
        kr_sb = qkv.tile([NR, T], BF, name="kr")
        v_sb = qkv.tile([P, T // P, NQ], BF, name="v")
        oT_sb = [qkv.tile([P, T], BF, name=f"oT{m}") for m in range(HLOC)]

        zpool = stk.enter_context(tc.tile_pool(name="p2z", bufs=2))
        rtmp = stk.enter_context(tc.tile_pool(name="p2t", bufs=2))
        apool = stk.enter_context(tc.tile_pool(name="p3s", bufs=2))
        ppool = stk.enter_context(tc.tile_pool(name="p3p", bufs=5))
        opool = stk.enter_context(tc.tile_pool(name="p3o", bufs=3))

        def proj(zt, nk, wt, mcol0, mcols, btile, ptag):
            ps = pst(ptag, [mcols, TLOC])
            for k in range(nk):
                nc.tensor.matmul(
                    ps, wt[:, k, mcol0:mcol0 + mcols], zt[:, k, :],
                    start=(k == 0), stop=(k == nk - 1 and btile is None))
            if btile is not None:
                nc.tensor.matmul(ps, btile[:, 0, mcol0:mcol0 + mcols],
                                 ones_tok[:], start=False, stop=True)
            return ps

        def rope_pair(zt, nk, wr, br, dst, csl, ptag_a, ptag_b):
            """dst[:, csl] = p*cos + rot(p)*sin, rot via Pi matmul."""
            psa = proj(zt, nk, wr, 0, NR, br, ptag_a)
            p_sb = rtmp.tile([NR, TLOC], BF, tag="p_sb", name="p_sb")
            nc.vector.tensor_copy(p_sb, psa)
            psb = pst(ptag_b, [NR, TLOC])
            nc.tensor.matmul(psb, pi_sb, p_sb, start=True, stop=True)
            t1 = rtmp.tile([NR, TLOC], BF, tag="t1", name="t1")
            nc.vector.tensor_mul(t1[:], p_sb[:], cos_sb[:, csl])
            t2 = rtmp.tile([NR, TLOC], BF, tag="t2", name="t2")
            nc.vector.tensor_mul(t2[:], psb[:], sin_sb[:, csl])
            nc.vector.tensor_add(dst[:, csl], t1[:], t2[:])

        # ------------- phase 2: kv path for all chunks -------------
        for c in range(NCH):
            csl = slice(c * TLOC, (c + 1) * TLOC)
            zkv = zpool.tile([P, KKV, TLOC], BF, tag="zkv", name="zkv")
            nc.sync.dma_start(
                zkv[:], ago_kv[c * KVR:(c + 1) * KVR, :]
                .rearrange("(k p) m -> p k m", p=P))
            spk = pst(f"sp{c % 2}", [P, 2 * TLOC])
            for m in range(HLOC):
                ps = spk[:, m * TLOC:(m + 1) * TLOC]
                for k in range(KKV):
                    nc.tensor.matmul(
                        ps, gku_t[:, k, m * P:(m + 1) * P], zkv[:, k, :],
                        start=(k == 0),
                        stop=(k == KKV - 1 and not has_bias))
                if has_bias:
                    nc.tensor.matmul(ps, bku_t[:, 0, m * P:(m + 1) * P],
                                     ones_tok[:], start=False, stop=True)
                nc.vector.tensor_copy(kc_sb[m][:, csl], ps)
            rope_pair(zkv, KKV, gkr_t,
                      bkr_t if has_bias else None, kr_sb, csl, "o0", "o1")
            for mt in range(MT):
                psv = pst("o1" if mt % 2 else "o0", [P, NQ])
                for k in range(KKV):
                    nc.tensor.matmul(
                        psv, zkv[:, k, mt * P:(mt + 1) * P], gvu_t[:, k, :],
                        start=(k == 0), stop=(k == KKV - 1 and not has_bias))
                if has_bias:
                    nc.tensor.matmul(psv, ones_row[:, :P], bvu_t[:, 0, :],
                                     start=False, stop=True)
                if mt % 2 == 0:
                    nc.vector.tensor_copy(v_sb[:, c * MT + mt, :], psv)
                else:
                    nc.scalar.activation(v_sb[:, c * MT + mt, :], psv, ACT.Copy)

        # ------------- q-path chunk emitters (interleaved into phase 3) ------
        def qpath_dma(c):
            # single sync-queue DMA: a second half on the scalar queue would
            # head-block the softmax exps behind it on its ring-buffer WAR
            zq = zpool.tile([P, KQ, TLOC], BF, tag="zq", name="zq")
            nc.sync.dma_start(
                zq[:], ago_q[c * QR:(c + 1) * QR, :]
                .rearrange("(k p) m -> p k m", p=P))
            return zq

        def qpath_pieces(c, zq):
            """Yield thunks: q up-proj + rope for chunk c, small PE pieces."""
            csl = slice(c * TLOC, (c + 1) * TLOC)

            def up(m):
                ps = proj(zq, KQ, gqu_t, m * P, P,
                          bqu_t if has_bias else None, "po")
                nc.vector.tensor_copy(qc_sb[m][:, csl], ps)

            rst = {}

            def rope_a():
                psa = proj(zq, KQ, gqr_t, 0, NR,
                           bqr_t if has_bias else None, "po")
                p_sb = rtmp.tile([NR, TLOC], BF, tag="p_sb", name="p_sb")
                nc.vector.tensor_copy(p_sb, psa)
                rst["p"] = p_sb

            def rope_b():
                p_sb = rst.pop("p")
                psb = pst("po", [NR, TLOC])
                nc.tensor.matmul(psb, pi_sb, p_sb, start=True, stop=True)
                t1 = rtmp.tile([NR, TLOC], BF, tag="t1", name="t1")
                nc.vector.tensor_mul(t1[:], p_sb[:], cos_sb[:, csl])
                t2 = rtmp.tile([NR, TLOC], BF, tag="t2", name="t2")
                nc.vector.tensor_mul(t2[:], psb[:], sin_sb[:, csl])
                nc.vector.tensor_add(qr_sb[:, csl], t1[:], t2[:])

            yield lambda: up(0)
            yield lambda: up(1)
            yield rope_a
            yield rope_b

        # ------------- phase 3: attention, software-pipelined -------------
        # chunk c loop body also emits: out-proj of chunk c-1, q-path of c+1
        def outproj_pieces(c):
            # fine-grained: one (token tile, d-column) piece per yield, so
            # the DVE casts never burst and starve the softmax acc adds
            tok0 = c * TLOC
            ots = {}
            for mt4 in range(MT):
                for nch in range(D // 512):
                    def run(mt4=mt4, nch=nch):
                        mt = (tok0 // P) + mt4
                        if nch == 0:
                            ots[mt4] = opool.tile([P, D], BF, tag="ot",
                                                  name="ot")
                        ot = ots[mt4]
                        po = pst("po", [P, 512])
                        for k2 in range(HLOC):
                            nc.tensor.matmul(
                                po,
                                oT_sb[k2][:, mt * P:(mt + 1) * P],
                                wout_t[:, k2, nch * 512:(nch + 1) * 512],
                                start=(k2 == 0), stop=(k2 == HLOC - 1))
                        # DVE only: ACT copies here would delay the exps
                        # queued behind them and stall the PV pipeline
                        nc.vector.tensor_copy(
                            ot[:, nch * 512:(nch + 1) * 512], po)
                        if nch == D // 512 - 1:
                            nc.gpsimd.dma_start(
                                out_p[mt * P:(mt + 1) * P, :], ot[:])
                    yield run

        def finish_pieces(c):
            """Softmax denominator + normalization for chunk c (after s-loop)."""
            tsl = slice(c * TLOC, (c + 1) * TLOC)
            for h in range(HLOC):
                def run(h=h):
                    den = pst("po", [1, TLOC])
                    nc.tensor.matmul(den, ones_col[:], acc[c % 2][h][:],
                                     start=True, stop=True)
                    denf = apool.tile([1, TLOC], F32, tag="denf", name="denf")
                    nc.vector.tensor_copy(denf, den)
                    rdf = apool.tile([1, TLOC], F32, tag="rdf", name="rdf")
                    rsc = apool.tile([1, TLOC], F32, tag="rsc", name="rsc")
                    nc.vector.reciprocal_approx_accurate(rdf[:], denf[:], rsc[:])
                    rb = apool.tile([P, TLOC], F32, tag=f"rb{h}", name="rb")
                    nc.gpsimd.partition_broadcast(rb[:], rdf[:])
                    nc.vector.tensor_mul(oT_sb[h][:, tsl], o_ps[c % 2][h][:],
                                         rb[:])
                yield run

        # double-buffered per-chunk state (chunk c vs c-1 overlap)
        acc = [[apool.tile([P, TLOC], BF, tag=f"acc{pp}{h}", name=f"acc{pp}{h}")
                for h in range(HLOC)] for pp in range(2)]
        o_ps = [None, None]

        zq_cur = qpath_dma(0)
        for piece in qpath_pieces(0, zq_cur):
            piece()

        for c in range(NCH):
            b, qch = divmod(c, S // TLOC)
            tok0 = c * TLOC
            tsl = slice(tok0, tok0 + TLOC)
            o_ps[c % 2] = [pst(f"o{h}", [P, TLOC]) for h in range(HLOC)]

            # queue of deferred emissions spread across the s-loop
            extra = []
            if c + 1 < NCH:
                zq_nxt = qpath_dma(c + 1)
                extra.extend(qpath_pieces(c + 1, zq_nxt))
            if c > 0:
                extra.extend(outproj_pieces(c - 1))

            pts = {}
            for s in range(NKT + 2):
                kt0 = b * S + s * P
                if s < NKT:
                    # one 2-bank score tile per step (both heads side by
                    # side) -> a single fused exp over 1024 columns
                    stp = pst(f"sp{s % 2}", [P, 2 * TLOC])
                    for h in range(HLOC):
                        nc.tensor.matmul(
                            stp[:, h * TLOC:(h + 1) * TLOC],
                            kc_sb[h][:, kt0:kt0 + P], qc_sb[h][:, tsl],
                            start=True, stop=False)
                    for h in range(HLOC):
                        nc.tensor.matmul(
                            stp[:, h * TLOC:(h + 1) * TLOC],
                            kr_sb[h * RD:(h + 1) * RD, kt0:kt0 + P],
                            qr_sb[h * RD:(h + 1) * RD, tsl],
                            start=False, stop=True)
                    pt = ppool.tile([P, 2 * TLOC], BF, tag="pt", name="pt")
                    nc.scalar.activation(pt[:], stp[:], ACT.Exp)
                    pts[s] = pt
                    for h in range(HLOC):
                        sl = pt[:, h * TLOC:(h + 1) * TLOC]
                        if s == 0:
                            nc.vector.tensor_copy(acc[c % 2][h][:], sl)
                        else:
                            nc.vector.tensor_add(acc[c % 2][h][:],
                                                 acc[c % 2][h][:], sl)
                # PV lagged two steps so exp(sp) is already done when it
                # issues -> no PE stall, weight loads stay hidden
                if s > 1:
                    sp = s - 2
                    ptp = pts.pop(sp)
                    for h in range(HLOC):
                        nc.tensor.matmul(
                            o_ps[c % 2][h],
                            v_sb[:, b * NKT + sp, h * P:(h + 1) * P],
                            ptp[:, h * TLOC:(h + 1) * TLOC],
                            start=(sp == 0), stop=(sp == NKT - 1))
                # spread deferred q-path / out-proj emissions across steps
                if s >= 2 and extra:
                    extra.pop(0)()
                if s >= 8 and extra:
                    extra.pop(0)()
            while extra:
                extra.pop(0)()
            for piece in finish_pieces(c):
                piece()

        for piece in outproj_pieces(NCH - 1):
            piece()

    nc.compile()
    return nc


_BUILD_CACHE = {}


def _get_nc(has_bias: bool):
    if has_bias not in _BUILD_CACHE:
        _BUILD_CACHE[has_bias] = build(has_bias)
    return _BUILD_CACHE[has_bias]


def _bf(a):
    return np.ascontiguousarray(a).astype(ml_dtypes.bfloat16)


def _prep_in_maps(x, Wq_down, q_gamma, q_beta, Wq_up, Wq_rope,
                  Wkv_down, kv_gamma, kv_beta, Wk_up, Wv_up, Wk_rope, Wout):
    x = np.asarray(x, dtype=np.float32)
    xT = np.ascontiguousarray(x.reshape(T, D).T)  # [D, T]

    # rope rotate-half permutation, per 64-dim head block (2 local heads)
    Pi1 = np.zeros((RD, RD), np.float32)
    for i in range(RD // 2):
        Pi1[RD // 2 + i, i] = -1.0
        Pi1[i, RD // 2 + i] = 1.0
    Pi = np.zeros((NR, NR), np.float32)
    Pi[:RD, :RD] = Pi1
    Pi[RD:, RD:] = Pi1

    # rope tables, feature-major, duplicated for the 2 local heads
    inv_freq = 1.0 / (10000.0 ** (np.arange(0, RD, 2, dtype=np.float32) / RD))
    pos = (np.arange(T) % S).astype(np.float32)
    freqs = pos[:, None] * inv_freq[None, :]          # [T, 32]
    emb = np.concatenate([freqs, freqs], axis=1)      # [T, 64]
    cosT = np.ascontiguousarray(np.cos(emb).T)        # [64, T]
    sinT = np.ascontiguousarray(np.sin(emb).T)
    cos2 = np.concatenate([cosT, cosT], axis=0)       # [128, T]
    sin2 = np.concatenate([sinT, sinT], axis=0)

    q_gamma = np.asarray(q_gamma, np.float32)
    q_beta = np.asarray(q_beta, np.float32)
    kv_gamma = np.asarray(kv_gamma, np.float32)
    kv_beta = np.asarray(kv_beta, np.float32)
    has_bias = bool(np.any(q_beta) or np.any(kv_beta))

    Wq_up_h = np.asarray(Wq_up, np.float32).reshape(QR, H, HD)
    Wq_rope_h = np.asarray(Wq_rope, np.float32).reshape(QR, H, RD)
    Wk_up_h = np.asarray(Wk_up, np.float32).reshape(KVR, H, HD)
    Wk_rope_h = np.asarray(Wk_rope, np.float32).reshape(KVR, H, RD)
    Wv_up_h = np.asarray(Wv_up, np.float32).reshape(KVR, H, HD)
    Wout_h = np.asarray(Wout, np.float32).reshape(H, HD, D)

    in_maps = []
    for c in range(NCORES):
        hs = slice(HLOC * c, HLOC * (c + 1))
        wq_up_s = Wq_up_h[:, hs].reshape(QR, NQ)
        wq_rope_s = Wq_rope_h[:, hs].reshape(QR, NR)
        wk_up_s = Wk_up_h[:, hs].reshape(KVR, NQ)
        wk_rope_s = Wk_rope_h[:, hs].reshape(KVR, NR)
        wv_up_s = Wv_up_h[:, hs].reshape(KVR, NQ)
        wout_s = Wout_h[hs].reshape(NQ, D)

        gq_up = q_gamma[:, None] * wq_up_s * SCALE
        gq_rope = q_gamma[:, None] * wq_rope_s * SCALE
        gk_up = kv_gamma[:, None] * wk_up_s
        gk_rope = kv_gamma[:, None] * wk_rope_s
        gv_up = kv_gamma[:, None] * wv_up_s

        m = {
            "xt": _bf(xT[:, c * TLOC:(c + 1) * TLOC]),
            "wq_down": _bf(Wq_down),
            "wkv_down": _bf(Wkv_down),
            "gq_up": _bf(gq_up),
            "gq_rope": _bf(gq_rope),
            "gk_up": _bf(gk_up),
            "gk_rope": _bf(gk_rope),
            "gv_up": _bf(gv_up),
            "wout": _bf(wout_s),
            "cos_t": _bf(cos2),
            "sin_t": _bf(sin2),
            "pi_t": _bf(Pi),
        }
        if has_bias:
            m["bq_up"] = _bf((q_beta @ wq_up_s * SCALE)[None, :])
            m["bq_rope"] = _bf((q_beta @ wq_rope_s * SCALE)[None, :])
            m["bk_up"] = _bf((kv_beta @ wk_up_s)[None, :])
            m["bk_rope"] = _bf((kv_beta @ wk_rope_s)[None, :])
            m["bv_up"] = _bf((kv_beta @ wv_up_s)[None, :])
        in_maps.append(m)
    return in_maps, has_bias


def kernel(**inputs):
    in_maps, has_bias = _prep_in_maps(**inputs)
    nc = _get_nc(has_bias)
    res = run_bass_kernel_spmd(nc, in_maps, list(range(NCORES)))
    out = res.results[0]["out_p"].astype(np.float32)
    for c in range(1, NCORES):
        out = out + res.results[c]["out_p"].astype(np.float32)
    return out.reshape(B, S, D)


# revision 66
# speedup vs baseline: 1.0217x; 1.0058x over previous
"""Multi-Head Latent Attention (MLA) forward pass on 8 Trainium2 NeuronCores.

Sharding: num_heads tensor-parallel (2 heads/core) for up-projections,
attention and out-proj; the low-rank down-projections + LayerNorm are
token-parallel (512 tokens/core) followed by on-device AllGathers of the
bf16 latents (kv first, overlapped with the q path). Per-core partial
outputs (out-proj with input-dim-sliced Wout) are summed on the host.

Scheduling: engines execute their queues in order, so the emission order
software-pipelines the work: attention chunk c's key-tile loop carries
the q-path projections for chunk c+1 and the out-projection of chunk
c-1, with a fixed PSUM bank map so phases don't serialize on bank reuse.
Rope's rotate-half branch is a single 128x128 permutation matmul instead
of a second full-rank projection.

Self-contained: hardcodes all shapes from the problem spec.
"""

from contextlib import ExitStack

import numpy as np
import ml_dtypes

import concourse.bass as bass
import concourse.mybir as mybir
import concourse.tile as tile
from concourse import bacc
from concourse.bass_utils import run_bass_kernel_spmd
from concourse.masks import make_identity

# ---- problem dimensions (hardcoded) ----
NCORES = 8
P = 128
B = 2
S = 2048           # sequence length
T = B * S          # total tokens = 4096
D = 2048           # d_model
QR = 1536          # q rank
KVR = 512          # kv rank
H = 16             # heads
HD = 128           # head dim (content)
RD = 64            # rope dim
HLOC = H // NCORES # heads per core = 2
TLOC = T // NCORES # tokens per core = 512
NQ = HLOC * HD     # 256 per-core content out dims
NR = HLOC * RD     # 128 per-core rope out dims
SCALE = (HD + RD) ** -0.5
LN_EPS = 1e-5

BF = mybir.dt.bfloat16
F32 = mybir.dt.float32
AX = mybir.AxisListType
OP = mybir.AluOpType
ACT = mybir.ActivationFunctionType

NKT = S // P       # 16 key tiles per sequence
KQ = QR // P       # 12
KKV = KVR // P     # 4
KX = D // P        # 16
MT = TLOC // P     # 4 token tiles per core
NCH = 8            # token chunks of 512 across T


def build(has_bias: bool):
    nc = bacc.Bacc("TRN2", target_bir_lowering=False, debug=False,
                   num_devices=NCORES, enable_asserts=False)

    def din(name, shape, dt=BF):
        return nc.dram_tensor(name, shape, dt, kind="ExternalInput").ap()

    xt = din("xt", [D, TLOC])
    wq_down = din("wq_down", [D, QR])
    wkv_down = din("wkv_down", [D, KVR])
    gq_up = din("gq_up", [QR, NQ])
    gq_rope = din("gq_rope", [QR, NR])
    gk_up = din("gk_up", [KVR, NQ])
    gk_rope = din("gk_rope", [KVR, NR])
    gv_up = din("gv_up", [KVR, NQ])
    wout = din("wout", [NQ, D])
    cos_t = din("cos_t", [NR, T])
    sin_t = din("sin_t", [NR, T])
    pi_t = din("pi_t", [NR, NR])
    if has_bias:
        bq_up = din("bq_up", [1, NQ])
        bq_rope = din("bq_rope", [1, NR])
        bk_up = din("bk_up", [1, NQ])
        bk_rope = din("bk_rope", [1, NR])
        bv_up = din("bv_up", [1, NQ])
    out_p = nc.dram_tensor("out_p", [T, D], BF, kind="ExternalOutput").ap()

    agi_kv = nc.dram_tensor("agi_kv", [KVR, TLOC], BF).ap()
    ago_kv = nc.dram_tensor("ago_kv", [NCORES * KVR, TLOC], BF,
                            addr_space="Shared").ap()
    # tiny warm-up collective: absorbs the ~40us first-collective barrier /
    # stream-setup cost under the down-projection compute
    agi_w = nc.dram_tensor("agi_w", [8, 8], BF).ap()
    ago_w = nc.dram_tensor("ago_w", [NCORES * 8, 8], BF,
                           addr_space="Shared").ap()
    agi_q = nc.dram_tensor("agi_q", [QR, TLOC], BF).ap()
    ago_q = nc.dram_tensor("ago_q", [NCORES * QR, TLOC], BF,
                           addr_space="Shared").ap()

    with tile.TileContext(nc) as tc, ExitStack() as stk:
        # ---------------- constants ----------------
        const = stk.enter_context(tc.tile_pool(name="const", bufs=1))
        ident = const.tile([P, P], BF)
        make_identity(nc, ident)
        ones_col = const.tile([P, 1], BF)
        nc.vector.memset(ones_col, 1.0)
        ones_tok = const.tile([1, TLOC], BF)
        nc.vector.memset(ones_tok, 1.0)
        ones_row = const.tile([1, P], BF)
        nc.vector.memset(ones_row, 1.0)
        eps_t = const.tile([P, 1], F32)
        nc.vector.memset(eps_t, LN_EPS)
        # warm-up collective first: the initial barrier then overlaps phase 1
        nc.gpsimd.collective_compute(
            "AllGather", OP.bypass,
            ins=[agi_w[:]], outs=[ago_w[:]],
            replica_groups=[list(range(NCORES))])
        # off the sync queue so x/w loads aren't head-blocked at startup
        pi_sb = const.tile([NR, NR], BF)
        nc.scalar.dma_start(pi_sb, pi_t)
        cos_sb = const.tile([NR, T], BF)
        nc.scalar.dma_start(cos_sb, cos_t)
        sin_sb = const.tile([NR, T], BF)
        nc.scalar.dma_start(sin_sb, sin_t)

        # PSUM bank map (8 banks):
        #   sp0,sp1     : paired score tiles, 2 banks each (both heads side
        #                 by side; also phase-1 down accum, kv-path k_up)
        #   o0,o1       : PV accumulators (also rope Pi rotation)
        #   po (bufs=2) : q-path proj + out-proj + denominators + transposes
        psum = stk.enter_context(tc.tile_pool(name="psum", bufs=1, space="PSUM"))
        PBUFS = {"po": 2}

        def pst(tag, shape=None, dt=F32):
            return psum.tile(shape or [P, TLOC], dt, tag=tag, name=tag,
                             bufs=PBUFS.get(tag, 1))

        # ------------- phase 1: down-proj + LN + transpose, kv first -------------
        with (
            tc.tile_pool(name="p1x", bufs=1) as xpool,
            tc.tile_pool(name="p1w", bufs=2) as wpool,
            tc.tile_pool(name="p1c", bufs=1) as cpool,
            tc.tile_pool(name="p1z", bufs=1) as zpool,
            tc.tile_pool(name="p1s", bufs=2) as spool,
        ):
            x_all = xpool.tile([P, KX, TLOC], BF)
            xr = xt.rearrange("(k p) m -> p k m", p=P)

            def x_quarter(q4):
                nc.sync.dma_start(x_all[:, q4 * 4:(q4 + 1) * 4, :],
                                  xr[:, q4 * 4:(q4 + 1) * 4, :])

            z_kv = zpool.tile([P, KKV, TLOC], BF)
            z_q = zpool.tile([P, KQ, TLOC], BF)
            kv_ct = [cpool.tile([P, KVR], F32, name=f"ckv{m}") for m in range(MT)]
            q_ct = [cpool.tile([P, QR], F32, name=f"cq{m}") for m in range(MT)]


            def down_mms(wdram, ctiles, nch, with_x=False):
                w_all = wpool.tile([P, KX, 512], BF, tag="w")
                wr = wdram[:, nch * 512:(nch + 1) * 512].rearrange(
                    "(k p) n -> p k n", p=P)
                for q4 in range(4):
                    if with_x:  # interleave so the k=0 matmul starts earliest
                        x_quarter(q4)
                    nc.sync.dma_start(w_all[:, q4 * 4:(q4 + 1) * 4, :],
                                      wr[:, q4 * 4:(q4 + 1) * 4, :])
                # m-outer so each token tile's result (and its LayerNorm)
                # is ready a quarter-chunk earlier; copies on DVE only so the
                # Scalar queue flows straight to the LayerNorm -> AllGather
                sps = [pst(f"sp{i}", [P, 2 * TLOC]) for i in range(2)]
                for m in range(MT):
                    ps = sps[m // 2][:, (m % 2) * 512:(m % 2 + 1) * 512]
                    for k in range(KX):
                        nc.tensor.matmul(
                            ps, x_all[:, k, m * P:(m + 1) * P],
                            w_all[:, k, :],
                            start=(k == 0), stop=(k == KX - 1))
                    nc.vector.tensor_copy(
                        ctiles[m][:, nch * 512:(nch + 1) * 512], ps)

            def layer_norm_m(rank, cb):
                """LN for one 128-token tile; returns the normalized bf16 cn."""
                sumt = spool.tile([P, 1], F32, tag="sumt")
                nc.vector.reduce_sum(sumt, cb[:], axis=AX.X)
                sqs = spool.tile([P, rank], BF, tag="sqs")
                ss = spool.tile([P, 1], F32, tag="ss")
                nc.scalar.activation(sqs[:], cb[:], ACT.Square,
                                     accum_out=ss[:])
                mu = spool.tile([P, 1], F32, tag="mu")
                nc.vector.tensor_scalar_mul(mu, sumt, 1.0 / rank)
                musq = spool.tile([P, 1], F32, tag="musq")
                nc.vector.tensor_mul(musq, mu, mu)
                var = spool.tile([P, 1], F32, tag="var")
                nc.vector.tensor_scalar_mul(var, ss, 1.0 / rank)
                nc.vector.tensor_sub(var, var, musq)
                sd = spool.tile([P, 1], F32, tag="sd")
                nc.scalar.activation(sd, var, ACT.Sqrt, bias=eps_t[:])
                inv = spool.tile([P, 1], F32, tag="inv")
                nc.vector.reciprocal(inv, sd)
                cn = spool.tile([P, rank], BF, tag="cn")
                nc.vector.tensor_scalar(
                    cn[:], cb[:], scalar1=mu, scalar2=inv,
                    op0=OP.subtract, op1=OP.mult)
                return cn

            def transpose_m(nk, cn, zdst, m):
                for f in range(nk):
                    tps = pst("po", [P, P], BF)
                    nc.tensor.transpose(tps, cn[:, f * P:(f + 1) * P], ident)
                    if f % 2 == 0:
                        nc.vector.tensor_copy(
                            zdst[:, f, m * P:(m + 1) * P], tps)
                    else:
                        nc.scalar.activation(
                            zdst[:, f, m * P:(m + 1) * P], tps, ACT.Copy)

            def down_finish(rank, ctiles, zdst):
                nk = rank // P
                for m in range(MT):
                    cn = layer_norm_m(rank, ctiles[m])
                    transpose_m(nk, cn, zdst, m)

            down_mms(wkv_down, kv_ct, 0, with_x=True)
            down_finish(KVR, kv_ct, z_kv)
            nc.scalar.dma_start(
                agi_kv.rearrange("(k p) m -> p k m", p=P), z_kv[:])
            nc.gpsimd.collective_compute(
                "AllGather", OP.bypass,
                ins=[agi_kv[:]], outs=[ago_kv[:]],
                replica_groups=[list(range(NCORES))])

            # q matmuls keep PE busy while kv LayerNorm/AllGather run
            down_mms(wq_down, q_ct, 0)
            down_mms(wq_down, q_ct, 1)
            down_mms(wq_down, q_ct, 2)
            down_finish(QR, q_ct, z_q)
            nc.scalar.dma_start(
                agi_q.rearrange("(k p) m -> p k m", p=P), z_q[:])
            nc.gpsimd.collective_compute(
                "AllGather", OP.bypass,
                ins=[agi_q[:]], outs=[ago_q[:]],
                replica_groups=[list(range(NCORES))])

        # ---------------- phase 2/3 tiles ----------------
        wu = stk.enter_context(tc.tile_pool(name="wu", bufs=1))
        qkv = stk.enter_context(tc.tile_pool(name="qkv", bufs=1))

        def load_w(dram, rows, cols):
            if rows < P:
                t = wu.tile([rows, 1, cols], BF, name="w_" + dram.tensor.name)
                nc.sync.dma_start(t[:, 0, :], dram[:, :])
                return t
            t = wu.tile([P, rows // P, cols], BF, name="w_" + dram.tensor.name)
            nc.sync.dma_start(t[:], dram.rearrange("(k p) n -> p k n", p=P))
            return t

        gqu_t = load_w(gq_up, QR, NQ)
        gqr_t = load_w(gq_rope, QR, NR)
        gku_t = load_w(gk_up, KVR, NQ)
        gkr_t = load_w(gk_rope, KVR, NR)
        gvu_t = load_w(gv_up, KVR, NQ)
        wout_t = load_w(wout, NQ, D)
        if has_bias:
            bqu_t = load_w(bq_up, 1, NQ)
            bqr_t = load_w(bq_rope, 1, NR)
            bku_t = load_w(bk_up, 1, NQ)
            bkr_t = load_w(bk_rope, 1, NR)
            bvu_t = load_w(bv_up, 1, NQ)

        qc_sb = [qkv.tile([P, T], BF, name=f"qc{m}") for m in range(HLOC)]
        kc_sb = [qkv.tile([P, T], BF, name=f"kc{m}") for m in range(HLOC)]
        qr_sb = qkv.tile([NR, T], BF, name="qr")
        kr_sb = qkv.tile([NR, T], BF, name="kr")
        v_sb = qkv.tile([P, T // P, NQ], BF, name="v")
        oT_sb = [qkv.tile([P, T], BF, name=f"oT{m}") for m in range(HLOC)]

        zpool = stk.enter_context(tc.tile_pool(name="p2z", bufs=2))
        rtmp = stk.enter_context(tc.tile_pool(name="p2t", bufs=2))
        apool = stk.enter_context(tc.tile_pool(name="p3s", bufs=2))
        ppool = stk.enter_context(tc.tile_pool(name="p3p", bufs=5))
        opool = stk.enter_context(tc.tile_pool(name="p3o", bufs=3))

        def proj(zt, nk, wt, mcol0, mcols, btile, ptag):
            ps = pst(ptag, [mcols, TLOC])
            for k in range(nk):
                nc.tensor.matmul(
                    ps, wt[:, k, mcol0:mcol0 + mcols], zt[:, k, :],
                    start=(k == 0), stop=(k == nk - 1 and btile is None))
            if btile is not None:
                nc.tensor.matmul(ps, btile[:, 0, mcol0:mcol0 + mcols],
                                 ones_tok[:], start=False, stop=True)
            return ps

        def rope_pair(zt, nk, wr, br, dst, csl, ptag_a, ptag_b):
            """dst[:, csl] = p*cos + rot(p)*sin, rot via Pi matmul."""
            psa = proj(zt, nk, wr, 0, NR, br, ptag_a)
            p_sb = rtmp.tile([NR, TLOC], BF, tag="p_sb", name="p_sb")
            nc.vector.tensor_copy(p_sb, psa)
            psb = pst(ptag_b, [NR, TLOC])
            nc.tensor.matmul(psb, pi_sb, p_sb, start=True, stop=True)
            t1 = rtmp.tile([NR, TLOC], BF, tag="t1", name="t1")
            nc.vector.tensor_mul(t1[:], p_sb[:], cos_sb[:, csl])
            t2 = rtmp.tile([NR, TLOC], BF, tag="t2", name="t2")
            nc.vector.tensor_mul(t2[:], psb[:], sin_sb[:, csl])
            nc.vector.tensor_add(dst[:, csl], t1[:], t2[:])

        # ------------- phase 2: kv path for all chunks -------------
        for c in range(NCH):
            csl = slice(c * TLOC, (c + 1) * TLOC)
            zkv = zpool.tile([P, KKV, TLOC], BF, tag="zkv", name="zkv")
            nc.sync.dma_start(
                zkv[:], ago_kv[c * KVR:(c + 1) * KVR, :]
                .rearrange("(k p) m -> p k m", p=P))
            spk = pst(f"sp{c % 2}", [P, 2 * TLOC])
            for m in range(HLOC):
                ps = spk[:, m * TLOC:(m + 1) * TLOC]
                for k in range(KKV):
                    nc.tensor.matmul(
                        ps, gku_t[:, k, m * P:(m + 1) * P], zkv[:, k, :],
                        start=(k == 0),
                        stop=(k == KKV - 1 and not has_bias))
                if has_bias:
                    nc.tensor.matmul(ps, bku_t[:, 0, m * P:(m + 1) * P],
                                     ones_tok[:], start=False, stop=True)
                nc.vector.tensor_copy(kc_sb[m][:, csl], ps)
            rope_pair(zkv, KKV, gkr_t,
                      bkr_t if has_bias else None, kr_sb, csl, "o0", "o1")
            for mt in range(MT):
                psv = pst("o1" if mt % 2 else "o0", [P, NQ])
                for k in range(KKV):
                    nc.tensor.matmul(
                        psv, zkv[:, k, mt * P:(mt + 1) * P], gvu_t[:, k, :],
                        start=(k == 0), stop=(k == KKV - 1 and not has_bias))
                if has_bias:
                    nc.tensor.matmul(psv, ones_row[:, :P], bvu_t[:, 0, :],
                                     start=False, stop=True)
                if mt % 2 == 0:
                    nc.vector.tensor_copy(v_sb[:, c * MT + mt, :], psv)
                else:
                    nc.scalar.activation(v_sb[:, c * MT + mt, :], psv, ACT.Copy)

        # ------------- q-path chunk emitters (interleaved into phase 3) ------
        def qpath_dma(c):
            # single sync-queue DMA: a second half on the scalar queue would
            # head-block the softmax exps behind it on its ring-buffer WAR
            zq = zpool.tile([P, KQ, TLOC], BF, tag="zq", name="zq")
            nc.sync.dma_start(
                zq[:], ago_q[c * QR:(c + 1) * QR, :]
                .rearrange("(k p) m -> p k m", p=P))
            return zq

        def qpath_pieces(c, zq):
            """Yield thunks: q up-proj + rope for chunk c, small PE pieces."""
            csl = slice(c * TLOC, (c + 1) * TLOC)

            def up(m):
                ps = proj(zq, KQ, gqu_t, m * P, P,
                          bqu_t if has_bias else None, "po")
                nc.vector.tensor_copy(qc_sb[m][:, csl], ps)

            rst = {}

            def rope_a():
                psa = proj(zq, KQ, gqr_t, 0, NR,
                           bqr_t if has_bias else None, "po")
                p_sb = rtmp.tile([NR, TLOC], BF, tag="p_sb", name="p_sb")
                nc.vector.tensor_copy(p_sb, psa)
                rst["p"] = p_sb

            def rope_b():
                p_sb = rst.pop("p")
                psb = pst("po", [NR, TLOC])
                nc.tensor.matmul(psb, pi_sb, p_sb, start=True, stop=True)
                t1 = rtmp.tile([NR, TLOC], BF, tag="t1", name="t1")
                nc.vector.tensor_mul(t1[:], p_sb[:], cos_sb[:, csl])
                t2 = rtmp.tile([NR, TLOC], BF, tag="t2", name="t2")
                nc.vector.tensor_mul(t2[:], psb[:], sin_sb[:, csl])
                nc.vector.tensor_add(qr_sb[:, csl], t1[:], t2[:])

            yield lambda: up(0)
            yield lambda: up(1)
            yield rope_a
            yield rope_b

        # ------------- phase 3: attention, software-pipelined -------------
        # chunk c loop body also emits: out-proj of chunk c-1, q-path of c+1
        def outproj_pieces(c):
            # fine-grained: one (token tile, d-column) piece per yield, so
            # the DVE casts never burst and starve the softmax acc adds
            tok0 = c * TLOC
            ots = {}
            for mt4 in range(MT):
                for nch in range(D // 512):
                    def run(mt4=mt4, nch=nch):
                        mt = (tok0 // P) + mt4
                        if nch == 0:
                            ots[mt4] = opool.tile([P, D], BF, tag="ot",
                                                  name="ot")
                        ot = ots[mt4]
                        po = pst("po", [P, 512])
                        for k2 in range(HLOC):
                            nc.tensor.matmul(
                                po,
                                oT_sb[k2][:, mt * P:(mt + 1) * P],
                                wout_t[:, k2, nch * 512:(nch + 1) * 512],
                                start=(k2 == 0), stop=(k2 == HLOC - 1))
                        # DVE only: ACT copies here would delay the exps
                        # queued behind them and stall the PV pipeline
                        nc.vector.tensor_copy(
                            ot[:, nch * 512:(nch + 1) * 512], po)
                        if nch == D // 512 - 1:
                            nc.gpsimd.dma_start(
                                out_p[mt * P:(mt + 1) * P, :], ot[:])
                    yield run

        def finish_pieces(c):
            """Softmax denominator + normalization for chunk c (after s-loop)."""
            tsl = slice(c * TLOC, (c + 1) * TLOC)
            for h in range(HLOC):
                def run(h=h):
                    den = pst("po", [1, TLOC])
                    nc.tensor.matmul(den, ones_col[:], acc[c % 2][h][:],
                                     start=True, stop=True)
                    denf = apool.tile([1, TLOC], F32, tag="denf", name="denf")
                    nc.vector.tensor_copy(denf, den)
                    rdf = apool.tile([1, TLOC], F32, tag="rdf", name="rdf")
                    rsc = apool.tile([1, TLOC], F32, tag="rsc", name="rsc")
                    nc.vector.reciprocal_approx_accurate(rdf[:], denf[:], rsc[:])
                    rb = apool.tile([P, TLOC], F32, tag=f"rb{h}", name="rb")
                    nc.gpsimd.partition_broadcast(rb[:], rdf[:])
                    nc.vector.tensor_mul(oT_sb[h][:, tsl], o_ps[c % 2][h][:],
                                         rb[:])
                yield run

        # double-buffered per-chunk state (chunk c vs c-1 overlap)
        acc = [[apool.tile([P, TLOC], BF, tag=f"acc{pp}{h}", name=f"acc{pp}{h}")
                for h in range(HLOC)] for pp in range(2)]
        o_ps = [None, None]

        zq_cur = qpath_dma(0)
        for piece in qpath_pieces(0, zq_cur):
            piece()

        for c in range(NCH):
            b, qch = divmod(c, S // TLOC)
            tok0 = c * TLOC
            tsl = slice(tok0, tok0 + TLOC)
            o_ps[c % 2] = [pst(f"o{h}", [P, TLOC]) for h in range(HLOC)]

            # queue of deferred emissions spread across the s-loop
            extra = []
            if c + 1 < NCH:
                zq_nxt = qpath_dma(c + 1)
                extra.extend(qpath_pieces(c + 1, zq_nxt))
            if c > 0:
                extra.extend(outproj_pieces(c - 1))

            pts = {}
            for s in range(NKT + 2):
                kt0 = b * S + s * P
                if s < NKT:
                    # one 2-bank score tile per step (both heads side by
                    # side) -> a single fused exp over 1024 columns
                    stp = pst(f"sp{s % 2}", [P, 2 * TLOC])
                    for h in range(HLOC):
                        nc.tensor.matmul(
                            stp[:, h * TLOC:(h + 1) * TLOC],
                            kc_sb[h][:, kt0:kt0 + P], qc_sb[h][:, tsl],
                            start=True, stop=False)
                    for h in range(HLOC):
                        nc.tensor.matmul(
                            stp[:, h * TLOC:(h + 1) * TLOC],
                            kr_sb[h * RD:(h + 1) * RD, kt0:kt0 + P],
                            qr_sb[h * RD:(h + 1) * RD, tsl],
                            start=False, stop=True)
                    pt = ppool.tile([P, 2 * TLOC], BF, tag="pt", name="pt")
                    nc.scalar.activation(pt[:], stp[:], ACT.Exp)
                    pts[s] = pt
                    for h in range(HLOC):
                        sl = pt[:, h * TLOC:(h + 1) * TLOC]
                        if s == 0:
                            nc.vector.tensor_copy(acc[c % 2][h][:], sl)
                        else:
                            nc.vector.tensor_add(acc[c % 2][h][:],
                                                 acc[c % 2][h][:], sl)
                # PV lagged two steps so exp(sp) is already done when it
                # issues -> no PE stall, weight loads stay hidden
                if s > 1:
                    sp = s - 2
                    ptp = pts.pop(sp)
                    for h in range(HLOC):
                        nc.tensor.matmul(
                            o_ps[c % 2][h],
                            v_sb[:, b * NKT + sp, h * P:(h + 1) * P],
                            ptp[:, h * TLOC:(h + 1) * TLOC],
                            start=(sp == 0), stop=(sp == NKT - 1))
                # spread deferred q-path / out-proj emissions across steps
                if s >= 2 and extra:
                    extra.pop(0)()
                if s >= 8 and extra:
                    extra.pop(0)()
            while extra:
                extra.pop(0)()
            for piece in finish_pieces(c):
                piece()

        for piece in outproj_pieces(NCH - 1):
            piece()

    nc.compile()
    return nc


_BUILD_CACHE = {}


def _get_nc(has_bias: bool):
    if has_bias not in _BUILD_CACHE:
        _BUILD_CACHE[has_bias] = build(has_bias)
    return _BUILD_CACHE[has_bias]


def _bf(a):
    return np.ascontiguousarray(a).astype(ml_dtypes.bfloat16)


def _prep_in_maps(x, Wq_down, q_gamma, q_beta, Wq_up, Wq_rope,
                  Wkv_down, kv_gamma, kv_beta, Wk_up, Wv_up, Wk_rope, Wout):
    x = np.asarray(x, dtype=np.float32)
    xT = np.ascontiguousarray(x.reshape(T, D).T)  # [D, T]

    # rope rotate-half permutation, per 64-dim head block (2 local heads)
    Pi1 = np.zeros((RD, RD), np.float32)
    for i in range(RD // 2):
        Pi1[RD // 2 + i, i] = -1.0
        Pi1[i, RD // 2 + i] = 1.0
    Pi = np.zeros((NR, NR), np.float32)
    Pi[:RD, :RD] = Pi1
    Pi[RD:, RD:] = Pi1

    # rope tables, feature-major, duplicated for the 2 local heads
    inv_freq = 1.0 / (10000.0 ** (np.arange(0, RD, 2, dtype=np.float32) / RD))
    pos = (np.arange(T) % S).astype(np.float32)
    freqs = pos[:, None] * inv_freq[None, :]          # [T, 32]
    emb = np.concatenate([freqs, freqs], axis=1)      # [T, 64]
    cosT = np.ascontiguousarray(np.cos(emb).T)        # [64, T]
    sinT = np.ascontiguousarray(np.sin(emb).T)
    cos2 = np.concatenate([cosT, cosT], axis=0)       # [128, T]
    sin2 = np.concatenate([sinT, sinT], axis=0)

    q_gamma = np.asarray(q_gamma, np.float32)
    q_beta = np.asarray(q_beta, np.float32)
    kv_gamma = np.asarray(kv_gamma, np.float32)
    kv_beta = np.asarray(kv_beta, np.float32)
    has_bias = bool(np.any(q_beta) or np.any(kv_beta))

    Wq_up_h = np.asarray(Wq_up, np.float32).reshape(QR, H, HD)
    Wq_rope_h = np.asarray(Wq_rope, np.float32).reshape(QR, H, RD)
    Wk_up_h = np.asarray(Wk_up, np.float32).reshape(KVR, H, HD)
    Wk_rope_h = np.asarray(Wk_rope, np.float32).reshape(KVR, H, RD)
    Wv_up_h = np.asarray(Wv_up, np.float32).reshape(KVR, H, HD)
    Wout_h = np.asarray(Wout, np.float32).reshape(H, HD, D)

    in_maps = []
    for c in range(NCORES):
        hs = slice(HLOC * c, HLOC * (c + 1))
        wq_up_s = Wq_up_h[:, hs].reshape(QR, NQ)
        wq_rope_s = Wq_rope_h[:, hs].reshape(QR, NR)
        wk_up_s = Wk_up_h[:, hs].reshape(KVR, NQ)
        wk_rope_s = Wk_rope_h[:, hs].reshape(KVR, NR)
        wv_up_s = Wv_up_h[:, hs].reshape(KVR, NQ)
        wout_s = Wout_h[hs].reshape(NQ, D)

        gq_up = q_gamma[:, None] * wq_up_s * SCALE
        gq_rope = q_gamma[:, None] * wq_rope_s * SCALE
        gk_up = kv_gamma[:, None] * wk_up_s
        gk_rope = kv_gamma[:, None] * wk_rope_s
        gv_up = kv_gamma[:, None] * wv_up_s

        m = {
            "xt": _bf(xT[:, c * TLOC:(c + 1) * TLOC]),
            "wq_down": _bf(Wq_down),
            "wkv_down": _bf(Wkv_down),
            "gq_up": _bf(gq_up),
            "gq_rope": _bf(gq_rope),
            "gk_up": _bf(gk_up),
            "gk_rope": _bf(gk_rope),
            "gv_up": _bf(gv_up),
            "wout": _bf(wout_s),
            "cos_t": _bf(cos2),
            "sin_t": _bf(sin2),
            "pi_t": _bf(Pi),
        }
        if has_bias:
            m["bq_up"] = _bf((q_beta @ wq_up_s * SCALE)[None, :])
            m["bq_rope"] = _bf((q_beta @ wq_rope_s * SCALE)[None, :])
            m["bk_up"] = _bf((kv_beta @ wk_up_s)[None, :])
            m["bk_rope"] = _bf((kv_beta @ wk_rope_s)[None, :])
            m["bv_up"] = _bf((kv_beta @ wv_up_s)[None, :])
        in_maps.append(m)
    return in_maps, has_bias


def kernel(**inputs):
    in_maps, has_bias = _prep_in_maps(**inputs)
    nc = _get_nc(has_bias)
    res = run_bass_kernel_spmd(nc, in_maps, list(range(NCORES)))
    out = res.results[0]["out_p"].astype(np.float32)
    for c in range(1, NCORES):
        out = out + res.results[c]["out_p"].astype(np.float32)
    return out.reshape(B, S, D)
